# revision 1
# baseline (speedup 1.0000x reference)
"""Trainium2 Bass kernel for BioBERT-ARG-GNN (gated pooling + 2-layer GCN + MLP head).

Strategy: pure data parallel over batch B=64 across 8 NeuronCores (8 graphs
per core).  All segment/gather ops are dense matmuls against one-hot
matrices built on-device from the index tensors (N=128 nodes == partition
dim).  GCN normalization (D^-1/2 (A+I) D^-1/2) factors into per-partition
scalings around a dense [128,128] adjacency matmul.  Matmul dtypes: f32r
(TF32-like, 1 cycle/row at free-dim>=256) for the big subtoken pooling,
bf16 for the [128,128] GCN matmuls (adjacency counts are exact), f32 for
the tiny FC head.  Phase split keeps each ACT function's table loaded once.
"""

import os
import sys

import numpy as np

for _p in ("/opt/trn_rl_repo", "/root/.axon_site/_ro/trn_rl_repo"):
    if os.path.isdir(_p) and _p not in sys.path:
        sys.path.insert(0, _p)

import ml_dtypes  # noqa: E402
import concourse.bass as bass  # noqa: E402
import concourse.mybir as mybir  # noqa: E402
from concourse import tile  # noqa: E402
from concourse.bass_utils import run_bass_kernel_spmd  # noqa: E402

# Problem shapes (hardcoded per contest rules).
B, S, H = 64, 512, 768
N, E = 128, 1024
GH, FH, L = 128, 256, 2
NCORES = 8
BL = B // NCORES  # graphs per core
SC = S // 128     # subtoken chunks per graph
EC = E // 128     # edge chunks per graph
HC = H // 128     # BERT-hidden chunks
FC = (H + GH) // 128  # concat-feature chunks for the FC head

f32 = mybir.dt.float32
f32r = mybir.dt.float32r
bf16 = mybir.dt.bfloat16
AFT = mybir.ActivationFunctionType
ALU = mybir.AluOpType
BF16 = ml_dtypes.bfloat16

_CACHE = {}


def _split_multi_waits(nc: bass.Bass) -> int:
    """Walrus in this container accepts one sync-wait per instruction; split
    extra waits into single-wait EventSemaphore nops just before it."""
    n_split = 0
    for fn in nc.m.functions:
        for blk in fn.blocks:
            new_instrs = []
            changed = False
            for inst in blk.instructions:
                si = getattr(inst, "sync_info", None)
                if si is not None and si.on_wait is not None and len(si.on_wait) > 1:
                    waits = list(si.on_wait)
                    for j, w in enumerate(waits[:-1]):
                        ev = mybir.InstEventSemaphore(
                            name=f"{inst.name}_ws{j}",
                            ins=[], outs=[],
                            engine=inst.engine,
                            sync_info=mybir.SyncInfo(on_wait=[w], on_update=[]),
                        )
                        new_instrs.append(ev)
                    inst.sync_info = mybir.SyncInfo(
                        on_wait=[waits[-1]], on_update=list(si.on_update))
                    n_split += 1
                    changed = True
                new_instrs.append(inst)
            if changed:
                blk.instructions = new_instrs
    return n_split


def build_program(br_val: float, b1_zero: bool, b2_zero: bool) -> bass.Bass:
    nc = bass.Bass()

    lh_d = nc.declare_dram_parameter("lh", [BL, S, H], f32r, isOutput=False)
    subv_d = nc.declare_dram_parameter("subv", [BL, 128, SC], f32, isOutput=False)
    esrc_d = nc.declare_dram_parameter("esrc", [BL, 128, EC], f32, isOutput=False)
    edst_d = nc.declare_dram_parameter("edst", [BL, 128, EC], f32, isOutput=False)
    wrb_d = nc.declare_dram_parameter("wrb", [128, H], f32, isOutput=False)
    w1t_d = nc.declare_dram_parameter("w1t", [128, HC, GH], bf16, isOutput=False)
    w2t_d = nc.declare_dram_parameter("w2t", [GH, GH], bf16, isOutput=False)
    wf1t_d = nc.declare_dram_parameter("wf1t", [128, FC, FH], f32, isOutput=False)
    wf2t_d = nc.declare_dram_parameter("wf2t", [128, 2, L], f32, isOutput=False)
    b1b_d = nc.declare_dram_parameter("b1b", [128, GH], f32, isOutput=False)
    b2b_d = nc.declare_dram_parameter("b2b", [128, GH], f32, isOutput=False)
    bf1b_d = nc.declare_dram_parameter("bf1b", [BL, FH], f32, isOutput=False)
    bf2b_d = nc.declare_dram_parameter("bf2b", [BL, L], f32, isOutput=False)
    iotaf_d = nc.declare_dram_parameter("iota_f", [128, 128], f32, isOutput=False)
    iota8_d = nc.declare_dram_parameter("iota8", [128, EC, 128], f32, isOutput=False)
    identb_d = nc.declare_dram_parameter("ident_b", [128, 128], bf16, isOutput=False)
    identf_d = nc.declare_dram_parameter("ident_f", [128, 128], f32, isOutput=False)
    onesr_d = nc.declare_dram_parameter("ones_r", [128, 1], f32r, isOutput=False)
    onesb_d = nc.declare_dram_parameter("ones_b", [128, 1], bf16, isOutput=False)
    meanb_d = nc.declare_dram_parameter("mean_b", [128, 1], bf16, isOutput=False)
    out_d = nc.declare_dram_parameter("out", [BL, L], f32, isOutput=True)

    with tile.TileContext(nc) as tc:
        with (
            tc.tile_pool(name="const", bufs=1) as cpool,
            tc.tile_pool(name="lhp", bufs=8) as lhpool,
            tc.tile_pool(name="scr", bufs=3) as scpool,
            tc.tile_pool(name="work", bufs=3) as wpool,
            tc.tile_pool(name="small", bufs=6) as spool,
            tc.tile_pool(name="psA", bufs=2, space="PSUM") as psA,
            tc.tile_pool(name="psB", bufs=2, space="PSUM") as psB,
            tc.tile_pool(name="psC", bufs=2, space="PSUM") as psC,
        ):
            # ---- early constants (ACT HWDGE ring; SP ring is reserved for lh) ----
            iota8 = cpool.tile([128, EC, 128], f32)
            nc.scalar.dma_start(iota8[:], iota8_d[:])
            ident_b = cpool.tile([128, 128], bf16)
            nc.scalar.dma_start(ident_b[:], identb_d[:])
            ones_b = cpool.tile([128, 1], bf16)
            nc.scalar.dma_start(ones_b[:], onesb_d[:])
            wrb = cpool.tile([128, H], f32)
            nc.scalar.dma_start(wrb[:], wrb_d[:])
            w1s = cpool.tile([128, HC, GH], bf16)
            nc.scalar.dma_start(w1s[:], w1t_d[:])
            w2s = cpool.tile([GH, GH], bf16)
            nc.scalar.dma_start(w2s[:], w2t_d[:])
            mean_b = cpool.tile([128, 1], bf16)
            nc.scalar.dma_start(mean_b[:], meanb_d[:])
            # pooled graph embeddings (written one column per graph)
            catT6 = cpool.tile([128, BL], f32)

            # ---------- phase 0: adjacency + degrees for all graphs ----------
            atis = []
            dinvs = []
            subvs = []
            for g in range(BL):
                subv = spool.tile([128, SC], f32, tag="subv", bufs=BL)
                nc.sync.dma_start(subv[:], subv_d[g])
                subvs.append(subv)
                esrc = spool.tile([128, EC], f32, tag="esrc", bufs=2)
                nc.sync.dma_start(esrc[:], esrc_d[g])
                edst = spool.tile([128, EC], f32, tag="edst", bufs=2)
                nc.sync.dma_start(edst[:], edst_d[g])

                at_ps = psB.tile([128, 128], f32, tag="mm")
                s_all = wpool.tile([128, EC, 128], bf16, tag="ohS")
                nc.vector.tensor_tensor(
                    out=s_all[:], in0=iota8[:],
                    in1=esrc[:].broadcast_to([128, EC, 128]), op=ALU.is_equal)
                d_all = wpool.tile([128, EC, 128], bf16, tag="ohD")
                nc.vector.tensor_tensor(
                    out=d_all[:], in0=iota8[:],
                    in1=edst[:].broadcast_to([128, EC, 128]), op=ALU.is_equal)
                for c in range(EC):
                    nc.tensor.matmul(at_ps[:], s_all[:, c, :], d_all[:, c, :],
                                     start=(c == 0), stop=False)
                # += I (self-loops) via identity outer product, exact in bf16
                nc.tensor.matmul(at_ps[:], ident_b[:], ident_b[:], start=False,
                                 stop=True)
                ati = wpool.tile([128, 128], bf16, tag="ati", bufs=BL)
                nc.scalar.copy(ati[:], at_ps[:])
                atis.append(ati)
                # deg[d] = sum_s ATI[s,d]  -> dinv = 1/sqrt(deg)
                deg_ps = psB.tile([128, 1], f32, tag="mm")
                nc.tensor.matmul(deg_ps[:], ati[:], ones_b[:],
                                 start=True, stop=True)
                sdeg = spool.tile([128, 1], f32, tag="sv")
                nc.scalar.activation(sdeg[:], deg_ps[:], AFT.Sqrt)
                dinv = spool.tile([128, 1], f32, tag="dinv", bufs=BL)
                nc.vector.reciprocal(dinv[:], sdeg[:])
                dinvs.append(dinv)

            # ---------- phase 1: gate + pooling + GCN per graph ----------
            _b1b = [None]
            _b2b = [None]
            for g in range(BL):
                subv = subvs[g]
                ati = atis[g]
                dinv = dinvs[g]

                cnt_ps = psC.tile([128, SC], f32, tag="cnt")
                nf_ps = psA.tile([128, H], f32, tag="nf")
                p_all = wpool.tile([128, SC, 128], bf16, tag="ohP")
                nc.vector.tensor_tensor(
                    out=p_all[:], in0=iota8[:, 0:SC, :],
                    in1=subv[:].broadcast_to([128, SC, 128]), op=ALU.is_equal)
                for c in range(SC):
                    lht = lhpool.tile([128, H], f32r, tag="lh")
                    nc.sync.dma_start(lht[:], lh_d[g, c * 128 : (c + 1) * 128, :])
                    scr = scpool.tile([128, H], bf16, tag="scr")
                    logits = spool.tile([128, 1], f32, tag="sv")
                    nc.vector.scalar_tensor_tensor(
                        scr[:], lht[:].bitcast(f32), 0.0, wrb[:], ALU.bypass,
                        ALU.mult, accum_out=logits[:])
                    gate = spool.tile([128, 1], f32, tag="sv")
                    nc.scalar.activation(gate[:], logits[:], AFT.Sigmoid,
                                         bias=float(br_val))
                    pg_t = wpool.tile([128, 128], f32r, tag="ohPg")
                    nc.scalar.mul(pg_t[:], p_all[:, c, :], gate[:])
                    nc.tensor.matmul(cnt_ps[:, c : c + 1], p_all[:, c, :],
                                     ones_b[:], start=True, stop=True)
                    # pooled node feats: nf[n,h] += Pg[s,n]^T lh[s,h]
                    nc.tensor.matmul(nf_ps[:, 0:512], pg_t[:], lht[:, 0:512],
                                     start=(c == 0), stop=(c == SC - 1))
                    nc.tensor.matmul(nf_ps[:, 512:H], pg_t[:], lht[:, 512:H],
                                     start=(c == 0), stop=(c == SC - 1))

                # 1/max(cnt,1); combined layer-1 row scale s1 = invc * dinv
                cnt1 = spool.tile([128, 1], f32, tag="sv")
                nc.vector.tensor_reduce(cnt1[:], cnt_ps[:], mybir.AxisListType.X,
                                        ALU.add)
                mx = spool.tile([128, 1], f32, tag="sv")
                nc.vector.tensor_scalar_max(mx[:], cnt1[:], 1.0)
                invc = spool.tile([128, 1], f32, tag="sv")
                nc.vector.reciprocal(invc[:], mx[:])
                s1 = spool.tile([128, 1], f32, tag="sv")
                nc.vector.tensor_tensor(s1[:], invc[:], dinv[:], ALU.mult)

                # scale rows by s1 while moving PSUM->SBUF (bf16 for layer 1)
                nf_sb = wpool.tile([128, H], bf16, tag="nfsb", bufs=2)
                nc.vector.tensor_scalar_mul(nf_sb[:], nf_ps[:], s1[:])
                # transpose to nfT chunks [h,n]
                nfs = wpool.tile([128, HC, GH], bf16, tag="nfs", bufs=2)
                for hc in range(HC):
                    tr_ps = psB.tile([128, 128], bf16, tag="mm")
                    nc.tensor.transpose(tr_ps[:], nf_sb[:, hc * 128 : (hc + 1) * 128],
                                        ident_b[:])
                    nc.any.tensor_copy(nfs[:, hc, :], tr_ps[:])

                # GCN layer 1: T2 = (s1*sums) @ W1  (scale pre-applied)
                t1_ps = psB.tile([128, GH], f32, tag="mm")
                for hc in range(HC):
                    nc.tensor.matmul(t1_ps[:], nfs[:, hc, :], w1s[:, hc, :],
                                     start=(hc == 0), stop=(hc == HC - 1))
                t2 = wpool.tile([128, GH], bf16, tag="t2")
                nc.any.tensor_copy(t2[:], t1_ps[:])
                z_ps = psB.tile([128, GH], f32, tag="mm")
                nc.tensor.matmul(z_ps[:], ati[:], t2[:], start=True, stop=True)
                x1 = wpool.tile([128, GH], bf16, tag="x1")
                if b1_zero:
                    # x1 = dinv * relu(z)  (valid since dinv > 0)
                    nc.vector.tensor_scalar(x1[:], z_ps[:], 0.0, dinv[:],
                                            ALU.max, ALU.mult)
                else:
                    if g == 0 and _b1b[0] is None:
                        _b1b[0] = cpool.tile([128, GH], f32, name="b1bt")
                        nc.scalar.dma_start(_b1b[0][:], b1b_d[:])
                    x1p = wpool.tile([128, GH], f32, tag="x1p")
                    nc.vector.scalar_tensor_tensor(x1p[:], z_ps[:], dinv[:],
                                                   _b1b[0][:], ALU.mult, ALU.add)
                    nc.vector.tensor_scalar_max(x1[:], x1p[:], 0.0)

                # GCN layer 2
                x1t_ps = psB.tile([128, GH], bf16, tag="mm")
                nc.tensor.transpose(x1t_ps[:], x1[:], ident_b[:])
                x1t = wpool.tile([128, GH], bf16, tag="x1t")
                nc.any.tensor_copy(x1t[:], x1t_ps[:])
                tp_ps = psB.tile([128, GH], f32, tag="mm")
                nc.tensor.matmul(tp_ps[:], x1t[:], w2s[:], start=True, stop=True)
                t2p = wpool.tile([128, GH], bf16, tag="t2")
                nc.vector.tensor_scalar_mul(t2p[:], tp_ps[:], dinv[:])
                z2_ps = psB.tile([128, GH], f32, tag="mm")
                nc.tensor.matmul(z2_ps[:], ati[:], t2p[:], start=True, stop=True)
                x2 = wpool.tile([128, GH], bf16, tag="x1")
                if b2_zero:
                    nc.vector.tensor_scalar(x2[:], z2_ps[:], 0.0, dinv[:],
                                            ALU.max, ALU.mult)
                else:
                    if g == 0 and _b2b[0] is None:
                        _b2b[0] = cpool.tile([128, GH], f32, name="b2bt")
                        nc.scalar.dma_start(_b2b[0][:], b2b_d[:])
                    x2p = wpool.tile([128, GH], f32, tag="x1p")
                    nc.vector.scalar_tensor_tensor(x2p[:], z2_ps[:], dinv[:],
                                                   _b2b[0][:], ALU.mult, ALU.add)
                    nc.vector.tensor_scalar_max(x2[:], x2p[:], 0.0)

                # graph mean pool -> column g of catT6
                pool_ps = psB.tile([128, 1], f32, tag="mm")
                nc.tensor.matmul(pool_ps[:], x2[:], mean_b[:], start=True,
                                 stop=True)
                nc.scalar.copy(catT6[:, g : g + 1], pool_ps[:])

            # ---------- FC head over all BL graphs ----------
            ident_f = cpool.tile([128, 128], f32)
            nc.scalar.dma_start(ident_f[:], identf_d[:])
            wf1s = cpool.tile([128, FC, FH], f32)
            nc.scalar.dma_start(wf1s[:], wf1t_d[:])
            wf2s = cpool.tile([128, 2, L], f32)
            nc.scalar.dma_start(wf2s[:], wf2t_d[:])
            bf1b = cpool.tile([BL, FH], f32)
            nc.scalar.dma_start(bf1b[:], bf1b_d[:])
            bf2b = cpool.tile([BL, L], f32)
            nc.scalar.dma_start(bf2b[:], bf2b_d[:])
            clsr = cpool.tile([BL, H], f32)
            nc.sync.dma_start(clsr[:], lh_d[:, 0, :].bitcast(f32))
            h1_ps = psB.tile([BL, FH], f32, tag="mm")
            for c in range(FC):
                if c < HC:
                    ct_ps = psB.tile([128, BL], f32, tag="mm")
                    nc.tensor.transpose(ct_ps[:], clsr[:, c * 128 : (c + 1) * 128],
                                        ident_f[0:BL, 0:BL])
                    catc = wpool.tile([128, BL], f32, tag="catc", bufs=2)
                    nc.any.tensor_copy(catc[:], ct_ps[:])
                else:
                    catc = catT6
                nc.tensor.matmul(h1_ps[:], catc[:], wf1s[:, c, :], start=(c == 0),
                                 stop=(c == FC - 1))
            h1s = wpool.tile([BL, FH], f32, tag="h1")
            nc.vector.scalar_tensor_tensor(h1s[:], h1_ps[:], 1.0, bf1b[:],
                                           ALU.bypass, ALU.add)
            hr = wpool.tile([BL, FH], f32, tag="h1")
            nc.vector.tensor_scalar_max(hr[:], h1s[:], 0.0)
            out_ps = psB.tile([BL, L], f32, tag="mm")
            for c in range(2):
                ht_ps = psB.tile([128, BL], f32, tag="mm")
                nc.tensor.transpose(ht_ps[:], hr[:, c * 128 : (c + 1) * 128],
                                    ident_f[0:BL, 0:BL])
                htc = wpool.tile([128, BL], f32, tag="catc", bufs=2)
                nc.any.tensor_copy(htc[:], ht_ps[:])
                nc.tensor.matmul(out_ps[:], htc[:], wf2s[:, c, :], start=(c == 0),
                                 stop=(c == 1))
            outs = wpool.tile([BL, L], f32, tag="outs")
            nc.vector.scalar_tensor_tensor(outs[:], out_ps[:], 1.0, bf2b[:],
                                           ALU.bypass, ALU.add)
            nc.sync.dma_start(out_d[:], outs[:])

    _split_multi_waits(nc)
    return nc


def _prepare_in_maps(inputs):
    lh = np.ascontiguousarray(np.asarray(inputs["last_hidden"], dtype=np.float32))
    submap = np.asarray(inputs["submap"]).astype(np.int64)
    edge_index = np.asarray(inputs["edge_index"]).astype(np.int64)
    assert lh.shape == (B, S, H)
    assert int(inputs.get("num_nodes", N)) == N

    wr = np.asarray(inputs["wr"], dtype=np.float32)
    br = float(np.asarray(inputs["br"], dtype=np.float32))
    W1 = np.asarray(inputs["W1"], dtype=np.float32)
    b1 = np.asarray(inputs["b1"], dtype=np.float32)
    W2 = np.asarray(inputs["W2"], dtype=np.float32)
    b2 = np.asarray(inputs["b2"], dtype=np.float32)
    Wf1 = np.asarray(inputs["Wf1"], dtype=np.float32)
    bf1 = np.asarray(inputs["bf1"], dtype=np.float32)
    Wf2 = np.asarray(inputs["Wf2"], dtype=np.float32)
    bf2 = np.asarray(inputs["bf2"], dtype=np.float32)

    # Shared (replicated) tensors.
    consts = {
        "wrb": np.ascontiguousarray(np.broadcast_to(wr, (128, H))),
        "w1t": np.ascontiguousarray(
            W1.reshape(HC, 128, GH).transpose(1, 0, 2)).astype(BF16),
        "w2t": np.ascontiguousarray(W2).astype(BF16),
        "wf1t": np.ascontiguousarray(
            Wf1.reshape(FC, 128, FH).transpose(1, 0, 2)),
        "wf2t": np.ascontiguousarray(
            Wf2.reshape(2, 128, L).transpose(1, 0, 2)),
        "b1b": np.ascontiguousarray(np.broadcast_to(b1, (128, GH))),
        "b2b": np.ascontiguousarray(np.broadcast_to(b2, (128, GH))),
        "bf1b": np.ascontiguousarray(np.broadcast_to(bf1, (BL, FH))),
        "bf2b": np.ascontiguousarray(np.broadcast_to(bf2, (BL, L))),
        "iota_f": np.ascontiguousarray(
            np.broadcast_to(np.arange(128, dtype=np.float32), (128, 128))),
        "iota8": np.ascontiguousarray(
            np.broadcast_to(np.arange(128, dtype=np.float32), (128, EC, 128))),
        "ident_b": np.eye(128, dtype=np.float32).astype(BF16),
        "ident_f": np.eye(128, dtype=np.float32),
        "ones_r": np.ones((128, 1), np.float32),
        "ones_b": np.ones((128, 1), np.float32).astype(BF16),
        "mean_b": np.full((128, 1), 1.0 / N, np.float32).astype(BF16),
    }

    # Per-graph index layouts: value of token t goes to partition t%128,
    # column t//128.
    subv = submap.reshape(B, SC, 128).transpose(0, 2, 1).astype(np.float32)
    esrc = edge_index[:, 0, :].reshape(B, EC, 128).transpose(0, 2, 1).astype(np.float32)
    edst = edge_index[:, 1, :].reshape(B, EC, 128).transpose(0, 2, 1).astype(np.float32)

    in_maps = []
    for i in range(NCORES):
        sl = slice(i * BL, (i + 1) * BL)
        m = dict(consts)
        m["lh"] = np.ascontiguousarray(lh[sl])
        m["subv"] = np.ascontiguousarray(subv[sl])
        m["esrc"] = np.ascontiguousarray(esrc[sl])
        m["edst"] = np.ascontiguousarray(edst[sl])
        in_maps.append(m)
    flags = (br, bool(np.all(b1 == 0)), bool(np.all(b2 == 0)))
    return in_maps, flags


def _run(inputs, trace=False):
    in_maps, flags = _prepare_in_maps(inputs)
    key = ("prog",) + flags
    if key not in _CACHE:
        _CACHE[key] = build_program(*flags)
    nc = _CACHE[key]
    res = run_bass_kernel_spmd(nc, in_maps, list(range(NCORES)), trace=trace)
    out = np.concatenate([np.asarray(res.results[i]["out"]) for i in range(NCORES)],
                         axis=0).astype(np.float32)
    return out, res


def kernel(**inputs) -> np.ndarray:
    out, _ = _run(inputs, trace=False)
    return out



# revision 6
# speedup vs baseline: 1.7217x; 1.7217x over previous
"""Trainium2 Bass kernel for BioBERT-ARG-GNN (gated pooling + 2-layer GCN + MLP head).

Strategy: pure data parallel over batch B=64 across 8 NeuronCores (8 graphs
per core).  All index-derived structure is precomputed on the host from the
int tensors (submap / edge_index):

  - P'[s,n]  : subtoken->node one-hot with 1/max(cnt,1) folded in (bf16)
  - Ahat[s,d]: dense normalized adjacency D^-1/2 (A+I) D^-1/2 (bf16)

and lh is host-cast to bf16 (halves HBM traffic, 3.5x faster matmuls vs
fp32 HIGH mode).  The dataflow is fully transpose-free by keeping pooled
features transposed (NFT[h,n] = sum_s lh[s,h]*g[s]*P'[s,n]) and
alternating matmul operand roles through the GCN:

  NFT -> T1[n,gh] -> Z1T[gh,d] -> T2[n,gh] -> Z2T[gh,d] -> pooled (ACT accum)

The per-token gate sigmoid(lh.wr+br) runs as one bf16 DVE product (2x mode)
plus per-chunk 4x-mode accumulate; sigmoid/scale/copies run on ACT.
Token layout s = 4p + c gives fully contiguous per-partition DMA.
"""

import os
import sys

import numpy as np

for _p in ("/opt/trn_rl_repo", "/root/.axon_site/_ro/trn_rl_repo"):
    if os.path.isdir(_p) and _p not in sys.path:
        sys.path.insert(0, _p)

import ml_dtypes  # noqa: E402
import concourse.bass as bass  # noqa: E402
import concourse.mybir as mybir  # noqa: E402
from concourse import tile  # noqa: E402
from concourse.bass_utils import run_bass_kernel_spmd  # noqa: E402

# Problem shapes (hardcoded per contest rules).
B, S, H = 64, 512, 768
N, E = 128, 1024
GH, FH, L = 128, 256, 2
NCORES = 8
BL = B // NCORES   # graphs per core
SC = S // 128      # token chunks per graph (s = 4p + c)
HC = H // 128      # BERT-hidden 128-chunks
FC = (H + GH) // 128  # concat-feature chunks for the FC head

# gd packing (per graph, per partition, bf16 elements)
OLH = 0            # [SC*H]  lh, token 4p+c at [c*H : (c+1)*H]
OPP = SC * H       # [SC*N]  P' one-hot * invc
OAT = OPP + SC * N  # [N]    Ahat[src=p, dst]
GDW = OAT + N

# cw packing (weights/constants, per partition, bf16 elements)
OWR = 0            # [H]     wr
OW1 = H            # [HC*GH] W1 tiled (hp, t*GH+gh) = W1[t*128+hp, gh]
OW2 = OW1 + HC * GH   # [GH] W2
OWF1 = OW2 + GH    # [FC*FH] Wf1 tiled, pooled block pre-divided by N
OCLS = OWF1 + FC * FH  # [HC*BL] cls^T
CWW = OCLS + HC * BL

f32 = mybir.dt.float32
bf16 = mybir.dt.bfloat16
AFT = mybir.ActivationFunctionType
ALU = mybir.AluOpType
BF16 = ml_dtypes.bfloat16

_CACHE = {}


def _split_multi_waits(nc: bass.Bass) -> int:
    """Walrus in this container accepts one sync-wait per instruction; split
    extra waits into single-wait EventSemaphore nops just before it."""
    n_split = 0
    for fn in nc.m.functions:
        for blk in fn.blocks:
            new_instrs = []
            changed = False
            for inst in blk.instructions:
                si = getattr(inst, "sync_info", None)
                if si is not None and si.on_wait is not None and len(si.on_wait) > 1:
                    waits = list(si.on_wait)
                    for j, w in enumerate(waits[:-1]):
                        ev = mybir.InstEventSemaphore(
                            name=f"{inst.name}_ws{j}",
                            ins=[], outs=[],
                            engine=inst.engine,
                            sync_info=mybir.SyncInfo(on_wait=[w], on_update=[]),
                        )
                        new_instrs.append(ev)
                    inst.sync_info = mybir.SyncInfo(
                        on_wait=[waits[-1]], on_update=list(si.on_update))
                    n_split += 1
                    changed = True
                new_instrs.append(inst)
            if changed:
                blk.instructions = new_instrs
    return n_split


def build_program(br_val: float, b1_zero: bool, b2_zero: bool,
                  bf1_zero: bool, bf2_zero: bool) -> bass.Bass:
    nc = bass.Bass()

    gd_d = nc.declare_dram_parameter("gd", [BL, 128, GDW], bf16, isOutput=False)
    cw_d = nc.declare_dram_parameter("cw", [128, CWW], bf16, isOutput=False)
    wf2r_d = nc.declare_dram_parameter("wf2r", [BL, L * FH], f32, isOutput=False)
    b1c_d = nc.declare_dram_parameter("b1c", [128, 1], f32, isOutput=False)
    b2c_d = nc.declare_dram_parameter("b2c", [128, 1], f32, isOutput=False)
    bf1r_d = nc.declare_dram_parameter("bf1r", [BL, FH], f32, isOutput=False)
    bf2r_d = nc.declare_dram_parameter("bf2r", [BL, L], f32, isOutput=False)
    out_d = nc.declare_dram_parameter("out", [BL, L], f32, isOutput=True)

    with tile.TileContext(nc) as tc:
        with (
            tc.tile_pool(name="const", bufs=1) as cpool,
            tc.tile_pool(name="gdp", bufs=BL) as gdpool,
            tc.tile_pool(name="scrp", bufs=2) as scrpool,
            tc.tile_pool(name="work", bufs=2) as wpool,
            tc.tile_pool(name="small", bufs=4) as spool,
            tc.tile_pool(name="psA", bufs=2, space="PSUM") as psA,
            tc.tile_pool(name="psB", bufs=2, space="PSUM") as psB,
            tc.tile_pool(name="psC", bufs=1, space="PSUM") as psC,
        ):
            # ---- DMAs (HWDGE on SP ring; consumers sem-wait via Tile) ----
            cw = cpool.tile([128, CWW], bf16)
            nc.sync.dma_start(cw[:], cw_d[:])
            gds = []
            for g in range(BL):
                gdt = gdpool.tile([128, GDW], bf16, tag="gd", bufs=BL)
                nc.sync.dma_start(gdt[:], gd_d[g])
                gds.append(gdt)
            wf2rs = cpool.tile([BL, L * FH], f32)
            nc.sync.dma_start(wf2rs[:], wf2r_d[:])
            b1cs = b2cs = bf1rs = bf2rs = None
            if not b1_zero:
                b1cs = cpool.tile([128, 1], f32)
                nc.sync.dma_start(b1cs[:], b1c_d[:])
            if not b2_zero:
                b2cs = cpool.tile([128, 1], f32)
                nc.sync.dma_start(b2cs[:], b2c_d[:])
            if not bf1_zero:
                bf1rs = cpool.tile([BL, FH], f32)
                nc.sync.dma_start(bf1rs[:], bf1r_d[:])
            if not bf2_zero:
                bf2rs = cpool.tile([BL, L], f32)
                nc.sync.dma_start(bf2rs[:], bf2r_d[:])

            # persistent scratch / accumulators
            pooledT = cpool.tile([128, BL], f32)
            scr2 = cpool.tile([128, H], bf16)      # TS-accum throwaway out
            x2scr = cpool.tile([128, GH], bf16)    # X2 relu throwaway out
            wr3 = cw[:, OWR:OWR + H].rearrange(
                "p (o h) -> p o h", o=1).broadcast_to([128, SC, H])

            for g in range(BL):
                gdt = gds[g]
                lh3 = gdt[:, OLH:OLH + SC * H].rearrange(
                    "p (c h) -> p c h", c=SC)
                ativ = gdt[:, OAT:OAT + N]

                # --- gate: logits = sum_h lh*wr (bf16 2x product + 4x accum)
                scr = scrpool.tile([128, SC, H], bf16, tag="scr")
                nc.vector.tensor_tensor(out=scr[:], in0=lh3, in1=wr3,
                                        op=ALU.mult)
                logits4 = spool.tile([128, SC], f32, tag="lg")
                for c in range(SC):
                    nc.vector.tensor_scalar(
                        scr2[:], scr[:, c, :], 0.0, None, ALU.bypass,
                        ALU.add, accum_out=logits4[:, c:c + 1])
                gate4 = spool.tile([128, SC], f32, tag="lg")
                if br_val == 0.0:
                    nc.scalar.activation(gate4[:], logits4[:], AFT.Sigmoid)
                else:
                    nc.scalar.activation(gate4[:], logits4[:], AFT.Sigmoid,
                                         bias=float(br_val))

                # --- Pg = P' * gate (ACT copy with per-partition scale)
                pg = wpool.tile([128, SC, N], bf16, tag="pg")
                for c in range(SC):
                    nc.scalar.activation(
                        pg[:, c, :], gdt[:, OPP + c * N:OPP + (c + 1) * N],
                        AFT.Copy, scale=gate4[:, c:c + 1])

                # --- pooled features, transposed: NFT[h,n] += lh_c,t^T @ Pg_c
                nft_ps = psA.tile([128, HC, GH], f32, tag="nft")
                for c in range(SC):
                    for t in range(HC):
                        nc.tensor.matmul(
                            nft_ps[:, t, :],
                            gdt[:, OLH + c * H + t * 128:
                                OLH + c * H + (t + 1) * 128],
                            pg[:, c, :],
                            start=(c == 0), stop=(c == SC - 1))
                nfts = wpool.tile([128, HC, GH], bf16, tag="nfts")
                nc.scalar.activation(nfts[:], nft_ps[:], AFT.Copy)

                # --- GCN chain in one PSUM bank: t1 | z1 | t2 | z2 slices
                gcn_ps = psB.tile([128, 4, GH], f32, tag="mm")
                t1_ps, z1_ps = gcn_ps[:, 0, :], gcn_ps[:, 1, :]
                t2_ps, z2_ps = gcn_ps[:, 2, :], gcn_ps[:, 3, :]
                # layer 1: T1[n,gh] = NF @ W1 ; Z1T[gh,d] = T1^T Ahat
                for t in range(HC):
                    nc.tensor.matmul(
                        t1_ps, nfts[:, t, :],
                        cw[:, OW1 + t * GH:OW1 + (t + 1) * GH],
                        start=(t == 0), stop=(t == HC - 1))
                t1s = spool.tile([128, GH], bf16, tag="t1", bufs=6)
                nc.scalar.activation(t1s[:], t1_ps, AFT.Copy)
                nc.tensor.matmul(z1_ps, t1s[:], ativ, start=True, stop=True)
                x1ts = spool.tile([128, GH], bf16, tag="t1", bufs=6)
                if b1_zero:
                    nc.scalar.activation(x1ts[:], z1_ps, AFT.Relu)
                else:
                    nc.scalar.activation(x1ts[:], z1_ps, AFT.Relu,
                                         bias=b1cs[:])

                # layer 2: T2[n,gh] = X1 @ W2 ; Z2T[gh,d] = T2^T Ahat
                nc.tensor.matmul(t2_ps, x1ts[:],
                                 cw[:, OW2:OW2 + GH], start=True, stop=True)
                t2s = spool.tile([128, GH], bf16, tag="t1", bufs=6)
                nc.scalar.activation(t2s[:], t2_ps, AFT.Copy)
                nc.tensor.matmul(z2_ps, t2s[:], ativ, start=True, stop=True)
                # relu + graph pool in one ACT op (sum over nodes = free axis)
                if b2_zero:
                    nc.scalar.activation(x2scr[:], z2_ps, AFT.Relu,
                                         accum_out=pooledT[:, g:g + 1])
                else:
                    nc.scalar.activation(x2scr[:], z2_ps, AFT.Relu,
                                         bias=b2cs[:],
                                         accum_out=pooledT[:, g:g + 1])

            # ---------- FC head over all BL graphs ----------
            pooledb = cpool.tile([128, BL], bf16)
            nc.scalar.activation(pooledb[:], pooledT[:], AFT.Copy)
            h1_ps = psC.tile([BL, FH], f32)
            for c in range(FC):
                if c < HC:
                    lhsTc = cw[:, OCLS + c * BL:OCLS + (c + 1) * BL]
                else:
                    lhsTc = pooledb[:]
                nc.tensor.matmul(h1_ps[:], lhsTc,
                                 cw[:, OWF1 + c * FH:OWF1 + (c + 1) * FH],
                                 start=(c == 0), stop=(c == FC - 1))
            hr = spool.tile([BL, FH], f32, tag="hr")
            if bf1_zero:
                nc.scalar.activation(hr[:], h1_ps[:], AFT.Relu)
            else:
                h1t = spool.tile([BL, FH], f32, tag="hr")
                nc.vector.tensor_tensor(out=h1t[:], in0=h1_ps[:],
                                        in1=bf1rs[:], op=ALU.add)
                nc.vector.tensor_scalar_max(hr[:], h1t[:], 0.0)
            outs = spool.tile([BL, L], f32, tag="out")
            scrF = spool.tile([BL, FH], f32, tag="hr")
            for l in range(L):
                nc.vector.scalar_tensor_tensor(
                    scrF[:], hr[:], 0.0, wf2rs[:, l * FH:(l + 1) * FH],
                    ALU.bypass, ALU.mult, accum_out=outs[:, l:l + 1])
            if not bf2_zero:
                outs2 = spool.tile([BL, L], f32, tag="out")
                nc.vector.tensor_tensor(out=outs2[:], in0=outs[:],
                                        in1=bf2rs[:], op=ALU.add)
                outs = outs2
            nc.sync.dma_start(out_d[:], outs[:])

    _split_multi_waits(nc)
    return nc


def _prepare_in_maps(inputs):
    lh = np.ascontiguousarray(np.asarray(inputs["last_hidden"], dtype=np.float32))
    submap = np.asarray(inputs["submap"]).astype(np.int64)
    edge = np.asarray(inputs["edge_index"]).astype(np.int64)
    assert lh.shape == (B, S, H)
    assert int(np.asarray(inputs.get("num_nodes", N))) == N

    wr = np.asarray(inputs["wr"], dtype=np.float32)
    br = float(np.asarray(inputs["br"], dtype=np.float32))
    W1 = np.asarray(inputs["W1"], dtype=np.float32)
    b1 = np.asarray(inputs["b1"], dtype=np.float32)
    W2 = np.asarray(inputs["W2"], dtype=np.float32)
    b2 = np.asarray(inputs["b2"], dtype=np.float32)
    Wf1 = np.asarray(inputs["Wf1"], dtype=np.float32)
    bf1 = np.asarray(inputs["bf1"], dtype=np.float32)
    Wf2 = np.asarray(inputs["Wf2"], dtype=np.float32)
    bf2 = np.asarray(inputs["bf2"], dtype=np.float32)

    # ---- host precompute of index-derived structure ----
    cnt = np.zeros((B, N), np.float32)
    np.add.at(cnt, (np.repeat(np.arange(B), S), submap.ravel()), 1.0)
    invc = 1.0 / np.maximum(cnt, 1.0)
    A = np.zeros((B, N, N), np.float32)
    np.add.at(A, (np.repeat(np.arange(B), E),
                  edge[:, 0, :].ravel(), edge[:, 1, :].ravel()), 1.0)
    A += np.eye(N, dtype=np.float32)[None]
    deg = A.sum(axis=1)           # in-degree incl self-loop (>= 1)
    dinv = 1.0 / np.sqrt(deg)
    ahat = A * dinv[:, :, None] * dinv[:, None, :]   # [B, src, dst]

    # token layout s = 4p + c
    lhr = lh.reshape(B, 128, SC * H)
    ppm = ((submap.reshape(B, 128, SC)[..., None] == np.arange(N))
           .astype(np.float32) * invc[:, None, None, :])
    gd = np.concatenate(
        [lhr, ppm.reshape(B, 128, SC * N), ahat], axis=2).astype(BF16)
    assert gd.shape == (B, 128, GDW)

    w1t = W1.reshape(HC, 128, GH).transpose(1, 0, 2).reshape(128, HC * GH)
    wf1m = Wf1.copy()
    wf1m[H:, :] /= N              # fold graph-mean 1/N into pooled block
    wf1t = wf1m.reshape(FC, 128, FH).transpose(1, 0, 2).reshape(128, FC * FH)
    wrb = np.broadcast_to(wr, (128, H))
    cw_base = np.concatenate([wrb, w1t, W2, wf1t], axis=1)

    wf2r = np.ascontiguousarray(
        np.broadcast_to(Wf2.T.reshape(1, L * FH), (BL, L * FH)))
    b1c = np.ascontiguousarray(b1[:, None])
    b2c = np.ascontiguousarray(b2[:, None])
    bf1r = np.ascontiguousarray(np.broadcast_to(bf1, (BL, FH)))
    bf2r = np.ascontiguousarray(np.broadcast_to(bf2, (BL, L)))

    in_maps = []
    for i in range(NCORES):
        sl = slice(i * BL, (i + 1) * BL)
        cls_core = lh[sl][:, 0, :]                      # [BL, H]
        clst = (cls_core.T.reshape(HC, 128, BL)
                .transpose(1, 0, 2).reshape(128, HC * BL))
        cwm = np.concatenate([cw_base, clst], axis=1).astype(BF16)
        assert cwm.shape == (128, CWW)
        in_maps.append({
            "gd": np.ascontiguousarray(gd[sl]),
            "cw": np.ascontiguousarray(cwm),
            "wf2r": wf2r,
            "b1c": b1c, "b2c": b2c, "bf1r": bf1r, "bf2r": bf2r,
        })
    flags = (br, bool(np.all(b1 == 0)), bool(np.all(b2 == 0)),
             bool(np.all(bf1 == 0)), bool(np.all(bf2 == 0)))
    return in_maps, flags


def _run(inputs, trace=False):
    in_maps, flags = _prepare_in_maps(inputs)
    key = ("prog",) + flags
    if key not in _CACHE:
        _CACHE[key] = build_program(*flags)
    nc = _CACHE[key]
    res = run_bass_kernel_spmd(nc, in_maps, list(range(NCORES)), trace=trace)
    out = np.concatenate([np.asarray(res.results[i]["out"]) for i in range(NCORES)],
                         axis=0).astype(np.float32)
    return out, res


def kernel(**inputs) -> np.ndarray:
    out, _ = _run(inputs, trace=False)
    return out


# revision 10
# speedup vs baseline: 1.7542x; 1.0189x over previous
"""Trainium2 Bass kernel for BioBERT-ARG-GNN (gated pooling + 2-layer GCN + MLP head).

Strategy: pure data parallel over batch B=64 across 8 NeuronCores (8 graphs
per core).  All index-derived structure is precomputed on the host from the
int tensors (submap / edge_index):

  - P'[s,n]  : subtoken->node one-hot with 1/max(cnt,1) folded in (bf16)
  - Ahat[s,d]: dense normalized adjacency D^-1/2 (A+I) D^-1/2 (bf16)

lh is host-cast to bf16 (halves HBM traffic, 3.5x faster matmuls vs fp32
HIGH mode).  The dataflow is fully transpose-free by keeping pooled
features transposed (NFT[h,n] = sum_s lh[s,h]*g[s]*P'[s,n]) and
alternating matmul operand roles through the GCN:

  NFT -> T1[n,gh] -> Z1T[gh,d] -> T2[n,gh2] -> Z2[d,gh2] -> pooled (PE matvec)

Engine balance: per-token gate logits run as fused STT+accum ops split
between DVE (chunks 0-1) and GpSimd (chunks 2-3); sigmoid + big copies on
ACT; single-matmul GCN stages use bf16 PSUM so their relu/copy moves run
on DVE in 2x mode.  Token layout s = 4p + c gives contiguous per-partition
DMA (one descriptor per partition per graph).
"""

import os
import sys

import numpy as np

for _p in ("/opt/trn_rl_repo", "/root/.axon_site/_ro/trn_rl_repo"):
    if os.path.isdir(_p) and _p not in sys.path:
        sys.path.insert(0, _p)

import ml_dtypes  # noqa: E402
import concourse.bass as bass  # noqa: E402
import concourse.mybir as mybir  # noqa: E402
from concourse import tile  # noqa: E402
from concourse.bass_utils import run_bass_kernel_spmd  # noqa: E402

# Problem shapes (hardcoded per contest rules).
B, S, H = 64, 512, 768
N, E = 128, 1024
GH, FH, L = 128, 256, 2
NCORES = 8
BL = B // NCORES   # graphs per core
SC = S // 128      # token chunks per graph (s = 4p + c)
HC = H // 128      # BERT-hidden 128-chunks
FC = (H + GH) // 128  # concat-feature chunks for the FC head

# gd packing (per graph, per partition, bf16 elements)
OLH = 0            # [SC*H]  lh, token 4p+c at [c*H : (c+1)*H]
OPP = SC * H       # [SC*N]  P' one-hot * invc
OAT = OPP + SC * N  # [N]    Ahat[src=p, dst]
GDW = OAT + N

# cw packing (weights/constants, per partition, bf16 elements)
OWR = 0            # [H]     wr
OW1 = H            # [HC*GH] W1 tiled (hp, t*GH+gh) = W1[t*128+hp, gh]
OW2 = OW1 + HC * GH   # [GH] W2
OWF1 = OW2 + GH    # [FC*FH] Wf1 tiled, pooled block pre-divided by N
OCLS = OWF1 + FC * FH  # [HC*BL] cls^T
OMC = OCLS + HC * BL   # [1]   1/N mean column
CWW = OMC + 1

f32 = mybir.dt.float32
bf16 = mybir.dt.bfloat16
AFT = mybir.ActivationFunctionType
ALU = mybir.AluOpType
BF16 = ml_dtypes.bfloat16

_CACHE = {}


def _split_multi_waits(nc: bass.Bass) -> int:
    """Walrus in this container accepts one sync-wait per instruction; split
    extra waits into single-wait EventSemaphore nops just before it."""
    n_split = 0
    for fn in nc.m.functions:
        for blk in fn.blocks:
            new_instrs = []
            changed = False
            for inst in blk.instructions:
                si = getattr(inst, "sync_info", None)
                if si is not None and si.on_wait is not None and len(si.on_wait) > 1:
                    waits = list(si.on_wait)
                    for j, w in enumerate(waits[:-1]):
                        ev = mybir.InstEventSemaphore(
                            name=f"{inst.name}_ws{j}",
                            ins=[], outs=[],
                            engine=inst.engine,
                            sync_info=mybir.SyncInfo(on_wait=[w], on_update=[]),
                        )
                        new_instrs.append(ev)
                    inst.sync_info = mybir.SyncInfo(
                        on_wait=[waits[-1]], on_update=list(si.on_update))
                    n_split += 1
                    changed = True
                new_instrs.append(inst)
            if changed:
                blk.instructions = new_instrs
    return n_split


def build_program(br_val: float, b1_zero: bool, b2_zero: bool,
                  bf1_zero: bool, bf2_zero: bool) -> bass.Bass:
    nc = bass.Bass()

    gd_d = nc.declare_dram_parameter("gd", [BL, 128, GDW], bf16, isOutput=False)
    cw_d = nc.declare_dram_parameter("cw", [128, CWW], bf16, isOutput=False)
    wf2r_d = nc.declare_dram_parameter("wf2r", [BL, L * FH], f32, isOutput=False)
    b1c_d = nc.declare_dram_parameter("b1c", [128, 1], f32, isOutput=False)
    b2c_d = nc.declare_dram_parameter("b2c", [128, 1], f32, isOutput=False)
    bf1r_d = nc.declare_dram_parameter("bf1r", [BL, FH], f32, isOutput=False)
    bf2r_d = nc.declare_dram_parameter("bf2r", [BL, L], f32, isOutput=False)
    out_d = nc.declare_dram_parameter("out", [BL, L], f32, isOutput=True)

    with tile.TileContext(nc) as tc:
        with (
            tc.tile_pool(name="const", bufs=1) as cpool,
            tc.tile_pool(name="gdp", bufs=BL) as gdpool,
            tc.tile_pool(name="work", bufs=2) as wpool,
            tc.tile_pool(name="small", bufs=4) as spool,
            tc.tile_pool(name="psA", bufs=2, space="PSUM") as psA,
            tc.tile_pool(name="psB", bufs=1, space="PSUM") as psB,
            tc.tile_pool(name="psC", bufs=1, space="PSUM") as psC,
        ):
            # ---- DMAs (HWDGE on SP ring; consumers sem-wait via Tile) ----
            cw = cpool.tile([128, CWW], bf16)
            nc.sync.dma_start(cw[:], cw_d[:])
            gds = []
            for g in range(BL):
                gdt = gdpool.tile([128, GDW], bf16, tag="gd", bufs=BL)
                nc.sync.dma_start(gdt[:], gd_d[g])
                gds.append(gdt)
            wf2rs = cpool.tile([BL, L * FH], f32)
            nc.sync.dma_start(wf2rs[:], wf2r_d[:])
            b1cs = b2cs = bf1rs = bf2rs = None
            if not b1_zero:
                b1cs = cpool.tile([128, 1], f32)
                nc.sync.dma_start(b1cs[:], b1c_d[:])
            if not b2_zero:
                b2cs = cpool.tile([128, 1], f32)
                nc.sync.dma_start(b2cs[:], b2c_d[:])
            if not bf1_zero:
                bf1rs = cpool.tile([BL, FH], f32)
                nc.sync.dma_start(bf1rs[:], bf1r_d[:])
            if not bf2_zero:
                bf2rs = cpool.tile([BL, L], f32)
                nc.sync.dma_start(bf2rs[:], bf2r_d[:])

            # persistent scratch / accumulators
            scr2d = cpool.tile([128, H], bf16)   # DVE STT throwaway out
            scr2a = cpool.tile([128, H], bf16)   # ACT reduce throwaway out
            x2scr = cpool.tile([128, GH], bf16)  # (b2 nonzero path)
            poolsb = cpool.tile([128, BL], f32) if not b2_zero else None
            wrv = cw[:, OWR:OWR + H]
            mcol = cw[:, OMC:OMC + 1]

            # psC: pooled columns [:, 0:BL] f32 + FC1 psum [0:8, 8:264]
            psc_t = psC.tile([128, 8 + FH], f32)
            pool_ps = psc_t[:, 0:BL]
            h1_ps = psc_t[0:BL, BL:BL + FH]

            for g in range(BL):
                gdt = gds[g]
                ativ = gdt[:, OAT:OAT + N]

                # --- gate logits: chunks 0-2 fused product+accum on DVE;
                # chunk 3 as GpSimd product + ACT free-axis accumulate
                logits4 = spool.tile([128, SC], f32, tag="lg")
                for c in range(3):
                    lhv = gdt[:, OLH + c * H:OLH + (c + 1) * H]
                    nc.vector.scalar_tensor_tensor(
                        scr2d[:], lhv, 0.0, wrv, ALU.bypass, ALU.mult,
                        accum_out=logits4[:, c:c + 1])
                prodg = wpool.tile([128, H], bf16, tag="prodg")
                nc.gpsimd.tensor_tensor(
                    out=prodg[:], in0=gdt[:, OLH + 3 * H:OLH + 4 * H],
                    in1=wrv, op=ALU.mult)
                nc.scalar.activation(scr2a[:], prodg[:], AFT.Copy,
                                     accum_out=logits4[:, 3:4])
                gate4 = spool.tile([128, SC], f32, tag="lg")
                if br_val == 0.0:
                    nc.scalar.activation(gate4[:], logits4[:], AFT.Sigmoid)
                else:
                    nc.scalar.activation(gate4[:], logits4[:], AFT.Sigmoid,
                                         bias=float(br_val))

                # --- Pg = P' * gate (GpSimd tensor_tensor, broadcast gate)
                pg = wpool.tile([128, SC, N], bf16, tag="pg")
                nc.gpsimd.tensor_tensor(
                    out=pg[:], in0=gdt[:, OPP:OPP + SC * N].rearrange(
                        "p (c n) -> p c n", c=SC),
                    in1=gate4[:].broadcast_to([128, SC, N]), op=ALU.mult)

                # --- pooled features, transposed: NFT[h,n] += lh_c,t^T @ Pg_c
                nft_ps = psA.tile([128, HC, GH], f32, tag="nft", bufs=2)
                for c in range(SC):
                    for t in range(HC):
                        nc.tensor.matmul(
                            nft_ps[:, t, :],
                            gdt[:, OLH + c * H + t * 128:
                                OLH + c * H + (t + 1) * 128],
                            pg[:, c, :],
                            start=(c == 0), stop=(c == SC - 1))
                nfts = wpool.tile([128, HC, GH], bf16, tag="nfts")
                nc.scalar.activation(nfts[:], nft_ps[:], AFT.Copy)

                # --- GCN layer 1: T1[n,gh] = NF @ W1 ; Z1T[gh,d] = T1^T Ahat
                t1_ps = psB.tile([128, GH], f32, tag="t1", bufs=1)
                for t in range(HC):
                    nc.tensor.matmul(
                        t1_ps[:], nfts[:, t, :],
                        cw[:, OW1 + t * GH:OW1 + (t + 1) * GH],
                        start=(t == 0), stop=(t == HC - 1))
                t1s = spool.tile([128, GH], bf16, tag="t1", bufs=6)
                nc.scalar.activation(t1s[:], t1_ps[:], AFT.Copy)
                zz_ps = psB.tile([128, 3, GH], f32, tag="zz", bufs=2)
                z1_ps, t2_ps, z2_ps = zz_ps[:, 0, :], zz_ps[:, 1, :], zz_ps[:, 2, :]
                nc.tensor.matmul(z1_ps, t1s[:], ativ, start=True, stop=True)
                x1ts = spool.tile([128, GH], bf16, tag="t1", bufs=6)
                if b1_zero:
                    nc.vector.tensor_scalar_max(x1ts[:], z1_ps, 0.0)
                else:
                    nc.scalar.activation(x1ts[:], z1_ps, AFT.Relu,
                                         bias=b1cs[:])

                # --- GCN layer 2
                nc.tensor.matmul(t2_ps, x1ts[:],
                                 cw[:, OW2:OW2 + GH], start=True, stop=True)
                t2s = spool.tile([128, GH], bf16, tag="t1", bufs=6)
                nc.vector.tensor_copy(t2s[:], t2_ps)
                if b2_zero:
                    # Z2[d,gh2]; relu on DVE; graph-mean via PE matvec column
                    nc.tensor.matmul(z2_ps, ativ, t2s[:], start=True, stop=True)
                    x2s = spool.tile([128, GH], bf16, tag="t1", bufs=6)
                    nc.vector.tensor_scalar_max(x2s[:], z2_ps, 0.0)
                    nc.tensor.matmul(pool_ps[:, g:g + 1], x2s[:], mcol,
                                     start=True, stop=True)
                else:
                    # Z2T[gh2,d]; relu+bias+free-axis pool accum on ACT
                    nc.tensor.matmul(z2_ps, t2s[:], ativ, start=True, stop=True)
                    nc.scalar.activation(x2scr[:], z2_ps, AFT.Relu,
                                         bias=b2cs[:],
                                         accum_out=poolsb[:, g:g + 1])

            # ---------- FC head over all BL graphs ----------
            pooledb = cpool.tile([128, BL], bf16)
            if b2_zero:
                nc.scalar.activation(pooledb[:], pool_ps, AFT.Copy)
            else:
                nc.scalar.activation(pooledb[:], poolsb[:], AFT.Copy)
            for c in range(FC):
                if c < HC:
                    lhsTc = cw[:, OCLS + c * BL:OCLS + (c + 1) * BL]
                else:
                    lhsTc = pooledb[:]
                nc.tensor.matmul(h1_ps, lhsTc,
                                 cw[:, OWF1 + c * FH:OWF1 + (c + 1) * FH],
                                 start=(c == 0), stop=(c == FC - 1))
            hr = spool.tile([BL, FH], f32, tag="hr")
            if bf1_zero:
                nc.scalar.activation(hr[:], h1_ps, AFT.Relu)
            else:
                h1t = spool.tile([BL, FH], f32, tag="hr")
                nc.vector.tensor_tensor(out=h1t[:], in0=h1_ps,
                                        in1=bf1rs[:], op=ALU.add)
                nc.vector.tensor_scalar_max(hr[:], h1t[:], 0.0)
            outs = spool.tile([BL, L], f32, tag="out")
            scrF = spool.tile([BL, FH], f32, tag="hr")
            for l in range(L):
                nc.vector.scalar_tensor_tensor(
                    scrF[:], hr[:], 0.0, wf2rs[:, l * FH:(l + 1) * FH],
                    ALU.bypass, ALU.mult, accum_out=outs[:, l:l + 1])
            if not bf2_zero:
                outs2 = spool.tile([BL, L], f32, tag="out")
                nc.vector.tensor_tensor(out=outs2[:], in0=outs[:],
                                        in1=bf2rs[:], op=ALU.add)
                outs = outs2
            nc.sync.dma_start(out_d[:], outs[:])

    _split_multi_waits(nc)
    return nc


def _prepare_in_maps(inputs):
    lh = np.ascontiguousarray(np.asarray(inputs["last_hidden"], dtype=np.float32))
    submap = np.asarray(inputs["submap"]).astype(np.int64)
    edge = np.asarray(inputs["edge_index"]).astype(np.int64)
    assert lh.shape == (B, S, H)
    assert int(np.asarray(inputs.get("num_nodes", N))) == N

    wr = np.asarray(inputs["wr"], dtype=np.float32)
    br = float(np.asarray(inputs["br"], dtype=np.float32))
    W1 = np.asarray(inputs["W1"], dtype=np.float32)
    b1 = np.asarray(inputs["b1"], dtype=np.float32)
    W2 = np.asarray(inputs["W2"], dtype=np.float32)
    b2 = np.asarray(inputs["b2"], dtype=np.float32)
    Wf1 = np.asarray(inputs["Wf1"], dtype=np.float32)
    bf1 = np.asarray(inputs["bf1"], dtype=np.float32)
    Wf2 = np.asarray(inputs["Wf2"], dtype=np.float32)
    bf2 = np.asarray(inputs["bf2"], dtype=np.float32)

    # ---- host precompute of index-derived structure ----
    cnt = np.zeros((B, N), np.float32)
    np.add.at(cnt, (np.repeat(np.arange(B), S), submap.ravel()), 1.0)
    invc = 1.0 / np.maximum(cnt, 1.0)
    A = np.zeros((B, N, N), np.float32)
    np.add.at(A, (np.repeat(np.arange(B), E),
                  edge[:, 0, :].ravel(), edge[:, 1, :].ravel()), 1.0)
    A += np.eye(N, dtype=np.float32)[None]
    deg = A.sum(axis=1)           # in-degree incl self-loop (>= 1)
    dinv = 1.0 / np.sqrt(deg)
    ahat = A * dinv[:, :, None] * dinv[:, None, :]   # [B, src, dst]

    # token layout s = 4p + c
    lhr = lh.reshape(B, 128, SC * H)
    ppm = ((submap.reshape(B, 128, SC)[..., None] == np.arange(N))
           .astype(np.float32) * invc[:, None, None, :])
    gd = np.concatenate(
        [lhr, ppm.reshape(B, 128, SC * N), ahat], axis=2).astype(BF16)
    assert gd.shape == (B, 128, GDW)

    w1t = W1.reshape(HC, 128, GH).transpose(1, 0, 2).reshape(128, HC * GH)
    wf1m = Wf1.copy()
    wf1m[H:, :] /= N              # fold graph-mean 1/N into pooled block
    wf1t = wf1m.reshape(FC, 128, FH).transpose(1, 0, 2).reshape(128, FC * FH)
    wrb = np.broadcast_to(wr, (128, H))
    mc = np.full((128, 1), 1.0 / N, np.float32)
    cw_base = np.concatenate([wrb, w1t, W2, wf1t], axis=1)

    wf2r = np.ascontiguousarray(
        np.broadcast_to(Wf2.T.reshape(1, L * FH), (BL, L * FH)))
    b1c = np.ascontiguousarray(b1[:, None])
    b2c = np.ascontiguousarray(b2[:, None])
    bf1r = np.ascontiguousarray(np.broadcast_to(bf1, (BL, FH)))
    bf2r = np.ascontiguousarray(np.broadcast_to(bf2, (BL, L)))

    in_maps = []
    for i in range(NCORES):
        sl = slice(i * BL, (i + 1) * BL)
        cls_core = lh[sl][:, 0, :]                      # [BL, H]
        clst = (cls_core.T.reshape(HC, 128, BL)
                .transpose(1, 0, 2).reshape(128, HC * BL))
        cwm = np.concatenate([cw_base, clst, mc], axis=1).astype(BF16)
        assert cwm.shape == (128, CWW)
        in_maps.append({
            "gd": np.ascontiguousarray(gd[sl]),
            "cw": np.ascontiguousarray(cwm),
            "wf2r": wf2r,
            "b1c": b1c, "b2c": b2c, "bf1r": bf1r, "bf2r": bf2r,
        })
    flags = (br, bool(np.all(b1 == 0)), bool(np.all(b2 == 0)),
             bool(np.all(bf1 == 0)), bool(np.all(bf2 == 0)))
    return in_maps, flags


def _run(inputs, trace=False):
    in_maps, flags = _prepare_in_maps(inputs)
    key = ("prog",) + flags
    if key not in _CACHE:
        _CACHE[key] = build_program(*flags)
    nc = _CACHE[key]
    res = run_bass_kernel_spmd(nc, in_maps, list(range(NCORES)), trace=trace)
    out = np.concatenate([np.asarray(res.results[i]["out"]) for i in range(NCORES)],
                         axis=0).astype(np.float32)
    return out, res


def kernel(**inputs) -> np.ndarray:
    out, _ = _run(inputs, trace=False)
    return out
